# revision 19
# baseline (speedup 1.0000x reference)
"""Trainium2 Bass kernel for nn_CLGD_86809878986948 (gnn_message_passing).

Strategy
--------
Shard the n (points) axis across 8 NeuronCores (256 points/core); both batches
are fused onto the free axis (512-point tiles), so all m-reductions stay local.
Host does cheap numpy prep: weight transposes for the VJP, layer-4 253->256
padding, folding the H/P projections into per-(b,m) bias vectors, summing the
duplicated dsx weight rows. Device computes the DeepSDF forward + VJP
(feature-major layout, lrelu via mask-multiply with biases folded into PSUM by
tiny K=2 matmuls) and the gated message passing (latent-major layout, running
max over m). Matmuls run in float32r (full PE rate at N=512).

Outputs (tuple, same as reference): (s, dsx, dsh, Gnew, dx, lp)
"""

import os

import numpy as np

P = 128
B = 2
M = 16
N = 2048
LAT = 256
NCORES = 8
NSH = N // NCORES      # points per core per batch
NPT = B * NSH          # point free-dim per core (b0|b1)
LEAK = 0.2

_CACHE = {}


# --------------------------------------------------------------------------
# host preprocessing
# --------------------------------------------------------------------------

def _prep(X, H, P_, G, params):
    f = np.float32
    W = [np.asarray(params[f"sdf_w{i}"], f) for i in range(9)]
    bb = [np.asarray(params[f"sdf_b{i}"], f) for i in range(9)]
    X = np.asarray(X, f)
    H = np.asarray(H, f)
    P_ = np.asarray(P_, f)
    G = np.asarray(G, f)
    Hb = H[:, 0, 0]                      # (B, 256)

    def padK(w, k=128):
        r = (-w.shape[0]) % k
        return np.pad(w, ((0, r), (0, 0))) if r else w

    def rne10(a):
        # hi-part rounding for the 3-pass fp32r matmuls; 10 bits survives the
        # hardware's fp32r storage rounding (~11 bits) exactly
        ab = np.asarray(a, f).view(np.uint32).astype(np.uint64)
        half = np.uint64(1 << 12)
        keep = np.uint64(0xFFFFFFFF) - np.uint64((1 << 13) - 1)
        return ((ab + half) & keep).astype(np.uint32).view(f)

    def hilo(w):
        hi = rne10(w)
        return hi, (np.asarray(w, f) - hi)

    d = {}
    # forward sdf weights (layout [K_in, out]); hi/lo split for exact-fp32 z
    W4p = np.concatenate([W[4][0:253], np.zeros((3, 512), f), W[4][253:512]], 0)
    for nm, w in (("w1", W[1]), ("w2", W[2]), ("w3", np.pad(W[3], ((0, 0), (0, 3)))),
                  ("w4a", W4p[0:256]), ("w5", W[5]), ("w6", W[6]), ("w7", W[7])):
        hi, lo = hilo(w)
        d[nm + "h"] = np.ascontiguousarray(hi)
        d[nm + "l"] = np.ascontiguousarray(lo)
    d["w8"] = W[8]                                   # [512, 1]
    d["w8c"] = np.ascontiguousarray(W[8][:, 0].reshape(4, 128).T)  # [128, 4]

    # biases (H projections folded); hi/lo rows [4, out] = [bh_b0, bh_b1, bl_b0, bl_b1]
    h1 = (Hb.astype(np.float64) @ W[0][0:256] + bb[0]).astype(f)   # (B, 512)
    h5 = (Hb.astype(np.float64) @ W[4][253:509] + bb[4]).astype(f)

    def bias4(rows2):  # (2, out) -> (4, out) hi/lo
        hi, lo = hilo(rows2)
        return np.concatenate([hi, lo], 0).astype(f)

    for i, nm in ((1, "bL2"), (2, "bL3"), (5, "bL6"), (6, "bL7"), (7, "bL8")):
        d[nm] = bias4(np.tile(bb[i][None, :], (2, 1)))   # [4, 512]
    d["bL4"] = bias4(np.tile(np.pad(bb[3], (0, 3))[None, :], (2, 1)))  # [4, 256]
    d["bL9"] = np.tile(bb[8][None, :], (2, 1))           # [2, 1]

    # L1 / L5-x packed lhsT blocks [128, 512]:
    #   xsd row map: 0:3 Xhi | 3 s | 4:7 dsx | 16:19 Xlo | 19:21 ind | 21:23 ind | 23:26 Xhi
    w0xh, w0xl = hilo(W[0][256:259])                 # [3, 512]
    b1h, b1l = hilo(h1)                              # [2, 512] each
    w0xpk = np.zeros((128, 512), f)
    w0xpk[0:3] = w0xh
    w0xpk[16:19] = w0xh
    w0xpk[23:26] = w0xl
    w0xpk[19:21] = b1h
    w0xpk[21:23] = b1l
    d["w0xpk"] = w0xpk
    w4xh, w4xl = hilo(W[4][509:512])
    b5h, b5l = hilo(h5)
    w4xpk = np.zeros((128, 512), f)
    w4xpk[0:3] = w4xh
    w4xpk[16:19] = w4xh
    w4xpk[23:26] = w4xl
    w4xpk[19:21] = b5h
    w4xpk[21:23] = b5l
    d["w4xpk"] = w4xpk

    # backward sdf weights (transposed, [K_out, in])
    for i in (1, 2, 5, 6, 7):
        d[f"w{i}T"] = np.ascontiguousarray(W[i].T)
    d["w4pT"] = np.pad(np.ascontiguousarray(W4p.T), ((0, 0), (0, 640 - 515)))  # [512,640]
    W3p = np.pad(W[3], ((0, 0), (0, 3)))
    d["w3pT"] = np.ascontiguousarray(W3p.T)          # [256, 512]
    W0T = np.ascontiguousarray(W[0].T)               # [512, 259]
    d["w0Th"] = np.ascontiguousarray(W0T[:, 0:256])  # [512, 256]
    d["w0Tx"] = np.pad(W0T[:, 256:259], ((0, 0), (0, 125)))  # [512, 128]

    # gate weights: feature order X(0:3) H(3:259) Pp(259:265) s(265) dsx(266:269) dsx(269:272) G(272:528)
    def gate_blocks(w):
        xsd = np.concatenate([w[0:3], w[265:266], w[266:269] + w[269:272]], 0)
        xsd = np.pad(xsd, ((0, 121), (0, 0)))        # [128, out]
        Gp = np.ascontiguousarray(w[272:528])
        PH = np.einsum("bmi,io->bmo", P_[:, :, 0, :], w[259:265]) + (Hb @ w[3:259])[:, None]
        return xsd.astype(f), Gp, PH.astype(f)       # PH: (B, 16, out)

    wr, wz, wg = (np.asarray(params[k], f) for k in ("wr", "wz", "wg"))
    wdx, wlp = np.asarray(params["wdx"], f), np.asarray(params["wlp"], f)
    d["wrx"], d["wrg"], rPH = gate_blocks(wr)
    d["wzx"], d["wzg"], zPH = gate_blocks(wz)
    d["wgx"], wggf, gPH = gate_blocks(wg)
    d["wgg16"] = wggf.astype(np.float16)
    dxx, dxg, dxPH = gate_blocks(wdx)
    lpx, lpg, lpPH = gate_blocks(wlp)
    d["wdlx"] = np.concatenate([dxx, lpx], 1)        # [128, 2]
    d["wdlg"] = np.concatenate([dxg, lpg], 1)        # [256, 2]
    for nm, PH in (("rPH", rPH), ("zPH", zPH), ("gPH", gPH)):
        lat_major = PH.transpose(2, 0, 1)            # (256, B, M)
        d[nm] = np.ascontiguousarray(
            lat_major.reshape(2, 128, B, M).transpose(1, 0, 2, 3))  # [128, 2, B, M]
    d["dlPH"] = np.ascontiguousarray(np.stack([dxPH[..., 0], lpPH[..., 0]], 0))  # [2, B, M]

    ind4 = np.zeros((4, NPT), f)
    ind4[0, 0:NSH] = 1.0
    ind4[1, NSH:] = 1.0
    ind4[2, 0:NSH] = 1.0
    ind4[3, NSH:] = 1.0
    d["ind4"] = ind4

    shards = []
    for c in range(NCORES):
        sl = slice(c * NSH, (c + 1) * NSH)
        xT = np.zeros((128, NPT), f)
        gT = np.empty((128, 2, NPT), f)
        for b in range(B):
            pts = slice(b * NSH, (b + 1) * NSH)
            xb = X[b, 0, sl, :].T                    # (3, NSH)
            xbh, xbl = hilo(xb)
            xT[0:3, pts] = xbh
            xT[16:19, pts] = xbl
            xT[23:26, pts] = xbh
            gblock = G[b, 0, sl, :].T                # (256, NSH)
            gT[:, 0, pts] = gblock[0:128]
            gT[:, 1, pts] = gblock[128:256]
        xT[19:21] = ind4[0:2]
        xT[21:23] = ind4[0:2]
        shards.append({"xT": np.ascontiguousarray(xT), "gT": np.ascontiguousarray(gT)})
    return d, shards


# --------------------------------------------------------------------------
# device kernel
# --------------------------------------------------------------------------

def _build():
    import concourse.bass as bass
    import concourse.mybir as mybir
    import concourse.tile as tile
    from concourse import bacc
    from contextlib import ExitStack

    f32 = mybir.dt.float32
    f32r = mybir.dt.float32r
    f16 = mybir.dt.float16
    AF = mybir.ActivationFunctionType
    OP = mybir.AluOpType
    AX = mybir.AxisListType

    nc = bacc.Bacc("TRN2", target_bir_lowering=False, debug=False, num_devices=NCORES)

    def din(name, shape, dt=f32r):
        return nc.dram_tensor(name, list(shape), dt, kind="ExternalInput")

    def dout(name, shape, dt=f32):
        return nc.dram_tensor(name, list(shape), dt, kind="ExternalOutput")

    # inputs
    t_xT = din("xT", [P, NPT])
    t_ind4 = din("ind4", [4, NPT])
    t_gT = din("gT", [P, 2, NPT])
    t_w = {}
    for nm, sh in (
        ("w0xpk", (P, 512)), ("w4xpk", (P, 512)),
        ("w1h", (512, 512)), ("w1l", (512, 512)), ("w2h", (512, 512)), ("w2l", (512, 512)),
        ("w3h", (512, 256)), ("w3l", (512, 256)), ("w4ah", (256, 512)), ("w4al", (256, 512)),
        ("w5h", (512, 512)), ("w5l", (512, 512)), ("w6h", (512, 512)), ("w6l", (512, 512)),
        ("w7h", (512, 512)), ("w7l", (512, 512)), ("w8", (512, 1)),
        ("w7T", (512, 512)), ("w6T", (512, 512)), ("w5T", (512, 512)),
        ("w4pT", (512, 640)), ("w3pT", (256, 512)), ("w2T", (512, 512)),
        ("w1T", (512, 512)), ("w0Th", (512, 256)), ("w0Tx", (512, 128)),
        ("bL2", (4, 512)), ("bL3", (4, 512)), ("bL4", (4, 256)),
        ("bL6", (4, 512)), ("bL7", (4, 512)), ("bL8", (4, 512)),
        ("bL9", (2, 1)),
        ("wrx", (P, 256)), ("wrg", (256, 256)), ("wzx", (P, 256)), ("wzg", (256, 256)),
        ("wgx", (P, 256)), ("wdlx", (P, 2)), ("wdlg", (256, 2)),
    ):
        t_w[nm] = din(nm, sh)
    t_w["wgg16"] = din("wgg16", [256, 256], f16)
    t_w8c = din("w8c", [P, 4], f32)
    t_rPH = din("rPH", [P, 2, B, M], f32)
    t_zPH = din("zPH", [P, 2, B, M], f32)
    t_gPH = din("gPH", [P, 2, B, M], f32)
    t_dlPH = din("dlPH", [2, B, M], f32)

    # outputs
    o_s = dout("s_out", [1, NPT], f32r)
    o_dsx = dout("dsx_out", [3, NPT], f32r)
    o_dsh = dout("dsh_out", [P, 2, B], f32)
    o_dx = dout("dx_out", [1, B], f32)
    o_lp = dout("lp_out", [M, NPT], f32)
    o_gnew = dout("gnew_out", [M, P, 2, NPT], f32r)

    with tile.TileContext(nc) as tc, ExitStack() as ctx:
        pers = ctx.enter_context(tc.tile_pool(name="pers", bufs=1))
        wpool = ctx.enter_context(tc.tile_pool(name="w", bufs=2))
        bpool = ctx.enter_context(tc.tile_pool(name="b", bufs=3))
        apool = ctx.enter_context(tc.tile_pool(name="a", bufs=2))
        lpool = ctx.enter_context(tc.tile_pool(name="alo", bufs=2))
        qpool = ctx.enter_context(tc.tile_pool(name="a32", bufs=2))
        kpool = ctx.enter_context(tc.tile_pool(name="mk32", bufs=3))
        gpool = ctx.enter_context(tc.tile_pool(name="g", bufs=2))
        mpool = ctx.enter_context(tc.tile_pool(name="m", bufs=1))
        tpool = ctx.enter_context(tc.tile_pool(name="t", bufs=3))
        pspool = ctx.enter_context(tc.tile_pool(name="ps", bufs=6, space="PSUM"))
        psdpool = ctx.enter_context(tc.tile_pool(name="psd", bufs=2, space="PSUM"))

        # ---- persistent setup ----
        xsd = pers.tile([P, NPT], f32r)
        nc.sync.dma_start(xsd[:], t_xT.ap())
        ind4 = pers.tile([4, NPT], f32r)
        nc.sync.dma_start(ind4[:], t_ind4.ap())

        def load_w(nm, kts, width):
            wt = wpool.tile([P, kts, width], f32r, tag="w")
            nc.sync.dma_start(
                wt[:, :, :], t_w[nm].ap().rearrange("(kt p) o -> p kt o", p=P))
            return wt

        def load_b(nm, width):
            bt = bpool.tile([2, width], f32r, tag="b")
            nc.sync.dma_start(bt[:], t_w[nm].ap())
            return bt

        # ---- forward SDF (hi/lo 3-pass for exact-fp32 pre-activations) ----
        def fwd_layer(ahi_in, alo_in, in_chunks, wname, bname, out_chunks, mtag,
                      extra=None):
            """Returns (ahi, alo [P,out_chunks,NPT] f32r, mask [..] f16).
            z = Whi.T@ahi + Whi.T@alo + Wlo.T@ahi + bias(hi/lo) [+ extra packed mm].
            extra: packed [128,512] lhsT tile multiplied against xsd."""
            wh = wpool.tile([P, in_chunks, out_chunks * P], f32r, tag="w")
            nc.sync.dma_start(
                wh[:], t_w[wname + "h"].ap().rearrange("(kt p) o -> p kt o", p=P))
            wl = wpool.tile([P, in_chunks, out_chunks * P], f32r, tag="w")
            nc.sync.dma_start(
                wl[:], t_w[wname + "l"].ap().rearrange("(kt p) o -> p kt o", p=P))
            if bname is not None:
                bt = bpool.tile([4, out_chunks * P], f32r, tag="b")
                nc.sync.dma_start(bt[:], t_w[bname].ap())
            ahi = apool.tile([P, 4, NPT], f32r, tag="a")
            alo = lpool.tile([P, 4, NPT], f32r, tag="alo")
            a32 = qpool.tile([P, 4, NPT], f32, tag="a32")
            mask = mpool.tile([P, out_chunks, NPT], f16, tag=mtag)
            for h in range(2):
                hs = slice(h * NSH, (h + 1) * NSH)
                for c in range(out_chunks):
                    cs = slice(c * P, (c + 1) * P)
                    ps = pspool.tile([P, NSH], f32, tag="ps")
                    if bname is not None:
                        nc.tensor.matmul(ps[:], bt[:, cs], ind4[:, hs],
                                         start=True, stop=False)
                    for kt in range(in_chunks):
                        nc.tensor.matmul(ps[:], wh[:, kt, cs], ahi_in[:, kt, hs],
                                         start=(bname is None and kt == 0), stop=False)
                        nc.tensor.matmul(ps[:], wh[:, kt, cs], alo_in[:, kt, hs],
                                         start=False, stop=False)
                        last = (kt == in_chunks - 1) and extra is None
                        nc.tensor.matmul(ps[:], wl[:, kt, cs], ahi_in[:, kt, hs],
                                         start=False, stop=last)
                    if extra is not None:
                        nc.tensor.matmul(ps[:], extra[:, cs], xsd[:, hs],
                                         start=False, stop=True)
                    nc.vector.tensor_scalar(mask[:, c, hs], ps[:], 0.0, LEAK,
                                            OP.is_gt, OP.max)
                    mk32 = kpool.tile([P, NSH], f32, tag="mk32")
                    nc.vector.tensor_scalar(mk32[:], ps[:], 0.0, LEAK,
                                            OP.is_gt, OP.max)
                    nc.vector.tensor_tensor(a32[:, c, hs], ps[:], mk32[:],
                                            OP.mult)
                    nc.scalar.copy(ahi[:, c, hs], a32[:, c, hs])
                    nc.gpsimd.tensor_tensor(alo[:, c, hs], a32[:, c, hs],
                                            ahi[:, c, hs], OP.subtract)
            return ahi, alo, mask

        # L1: single packed matmul per chunk (Xhi/Xlo/bias rows inside xsd)
        w0pk = pers.tile([P, 512], f32r)
        nc.sync.dma_start(w0pk[:], t_w["w0xpk"].ap())
        a1h = apool.tile([P, 4, NPT], f32r, tag="a")
        a1l = lpool.tile([P, 4, NPT], f32r, tag="alo")
        a1_32 = qpool.tile([P, 4, NPT], f32, tag="a32")
        m1 = mpool.tile([P, 4, NPT], f16, tag="m1")
        for h in range(2):
            hs = slice(h * NSH, (h + 1) * NSH)
            for c in range(4):
                cs = slice(c * P, (c + 1) * P)
                ps = pspool.tile([P, NSH], f32, tag="ps")
                nc.tensor.matmul(ps[:], w0pk[:, cs], xsd[:, hs], start=True, stop=True)
                nc.vector.tensor_scalar(m1[:, c, hs], ps[:], 0.0, LEAK,
                                        OP.is_gt, OP.max)
                mk32 = kpool.tile([P, NSH], f32, tag="mk32")
                nc.vector.tensor_scalar(mk32[:], ps[:], 0.0, LEAK,
                                        OP.is_gt, OP.max)
                nc.vector.tensor_tensor(a1_32[:, c, hs], ps[:], mk32[:], OP.mult)
                nc.scalar.copy(a1h[:, c, hs], a1_32[:, c, hs])
                nc.gpsimd.tensor_tensor(a1l[:, c, hs], a1_32[:, c, hs],
                                        a1h[:, c, hs], OP.subtract)

        a2h, a2l, m2 = fwd_layer(a1h, a1l, 4, "w1", "bL2", 4, "m2")
        a3h, a3l, m3 = fwd_layer(a2h, a2l, 4, "w2", "bL3", 4, "m3")
        a4h, a4l, m4 = fwd_layer(a3h, a3l, 4, "w3", "bL4", 2, "m4")
        w4pk = pers.tile([P, 512], f32r)
        nc.sync.dma_start(w4pk[:], t_w["w4xpk"].ap())
        a5h, a5l, m5 = fwd_layer(a4h, a4l, 2, "w4a", None, 4, "m5", extra=w4pk)
        a6h, a6l, m6 = fwd_layer(a5h, a5l, 4, "w5", "bL6", 4, "m6")
        a7h, a7l, m7 = fwd_layer(a6h, a6l, 4, "w6", "bL7", 4, "m7")
        a8h, a8l, m8 = fwd_layer(a7h, a7l, 4, "w7", "bL8", 4, "m8")
        a8 = a8h

        # layer 9: s = w8.T @ a8 + b8
        w8t = wpool.tile([P, 4, 1], f32r, tag="w8")
        nc.sync.dma_start(w8t[:], t_w["w8"].ap().rearrange("(kt p) o -> p kt o", p=P))
        b9t = load_b("bL9", 1)
        ps_s = psdpool.tile([1, NPT], f32, tag="psd")
        nc.tensor.matmul(ps_s[:], b9t[:], ind4[0:2, :], start=True, stop=False)
        for kt in range(4):
            nc.tensor.matmul(ps_s[:], w8t[:, kt, :], a8[:, kt, :],
                             start=False, stop=(kt == 3))
        s_sb = pers.tile([1, NPT], f32r)
        nc.vector.tensor_copy(s_sb[:], ps_s[:])
        nc.sync.dma_start(o_s.ap(), s_sb[:])
        nc.sync.dma_start(xsd[3:4, :], s_sb[:])

        # ---- backward SDF ----
        w8c_sb = pers.tile([P, 4], f32)
        nc.sync.dma_start(w8c_sb[:], t_w8c.ap())
        g8 = gpool.tile([P, 4, NPT], f32r, tag="g")
        for c in range(4):
            nc.vector.tensor_scalar(g8[:, c, :], m8[:, c, :], w8c_sb[:, c:c + 1],
                                    None, OP.mult)

        def bwd_layer(g_in, in_chunks, wTname, out_chunks, mask):
            wt = load_w(wTname, in_chunks, out_chunks * P)
            g_out = gpool.tile([P, 4, NPT], f32r, tag="g")
            for h in range(2):
                hs = slice(h * NSH, (h + 1) * NSH)
                for c in range(out_chunks):
                    cs = slice(c * P, (c + 1) * P)
                    ps = pspool.tile([P, NSH], f32, tag="ps")
                    for kt in range(in_chunks):
                        nc.tensor.matmul(ps[:], wt[:, kt, cs], g_in[:, kt, hs],
                                         start=(kt == 0), stop=(kt == in_chunks - 1))
                    nc.vector.tensor_tensor(g_out[:, c, hs], ps[:], mask[:, c, hs],
                                            OP.mult)
            return g_out

        g7 = bwd_layer(g8, 4, "w7T", 4, m7)
        g6 = bwd_layer(g7, 4, "w6T", 4, m6)
        g5 = bwd_layer(g6, 4, "w5T", 4, m5)
        # layer-5 bwd: g_a4c = w4pT.T-chunks @ g5 -> 5 chunks [a4p(2) | Hdir(2) | Xdir]
        w4pT_t = load_w("w4pT", 4, 640)
        g4 = gpool.tile([P, 4, NPT], f32r, tag="g")
        hd = pers.tile([P, 2, B], f32)
        xdir = pers.tile([3, NPT], f32)
        for c in range(5):
            cs = slice(c * P, (c + 1) * P)
            ps = pspool.tile([P, NPT], f32, tag="ps")
            for kt in range(4):
                nc.tensor.matmul(ps[:], w4pT_t[:, kt, cs], g5[:, kt, :],
                                 start=(kt == 0), stop=(kt == 3))
            if c < 2:
                nc.vector.tensor_tensor(g4[:, c, :], ps[:], m4[:, c, :], OP.mult)
            elif c < 4:
                nc.vector.reduce_sum(hd[:, c - 2, :],
                                     ps[:].rearrange("p (b n) -> p b n", b=B),
                                     axis=AX.X)
            else:
                nc.vector.tensor_copy(xdir[:], ps[0:3, :])
        g3 = bwd_layer(g4, 2, "w3pT", 4, m3)
        g2 = bwd_layer(g3, 4, "w2T", 4, m2)
        g1 = bwd_layer(g2, 4, "w1T", 4, m1)

        # dsx = (w0Tx.T-chunks @ g1)[0:3] + xdir
        w0Tx_t = load_w("w0Tx", 4, 128)
        ps_x = pspool.tile([P, NPT], f32, tag="ps")
        for kt in range(4):
            nc.tensor.matmul(ps_x[:], w0Tx_t[:, kt, :], g1[:, kt, :],
                             start=(kt == 0), stop=(kt == 3))
        dsx_sb = pers.tile([3, NPT], f32r)
        nc.vector.tensor_tensor(dsx_sb[:], ps_x[0:3, :], xdir[:], OP.add)
        nc.sync.dma_start(o_dsx.ap(), dsx_sb[:])
        nc.sync.dma_start(xsd[4:7, :], dsx_sb[:])

        # dsh: reduce g1 over points per batch, then small matmuls
        gred = pers.tile([P, 4, B], f32r)
        with nc.allow_low_precision(reason="f32r rounding of point-reduced sums"):
            for kt in range(4):
                nc.vector.reduce_sum(gred[:, kt, :],
                                     g1[:, kt, :].rearrange("p (b n) -> p b n", b=B),
                                     axis=AX.X)
        w0Th_t = load_w("w0Th", 4, 256)
        dsh_sb = pers.tile([P, 2, B], f32)
        for c in range(2):
            cs = slice(c * P, (c + 1) * P)
            psh = psdpool.tile([P, B], f32, tag="psd")
            for kt in range(4):
                nc.tensor.matmul(psh[:], w0Th_t[:, kt, cs], gred[:, kt, :],
                                 start=(kt == 0), stop=(kt == 3))
            nc.vector.tensor_tensor(dsh_sb[:, c, :], psh[:], hd[:, c, :], OP.add)
        nc.sync.dma_start(o_dsh.ap(), dsh_sb[:])

        # ---- phase 2: gated message passing (lat-major) ----
        gTsb = pers.tile([P, 2, NPT], f32r)
        nc.sync.dma_start(gTsb[:], t_gT.ap())
        gate_w = {}
        for nm in ("wrx", "wzx", "wgx"):
            wt = pers.tile([P, 256], f32r, tag=nm)
            nc.sync.dma_start(wt[:], t_w[nm].ap())
            gate_w[nm] = wt
        for nm in ("wrg", "wzg"):
            wt = pers.tile([P, 2, 256], f32r, tag=nm)
            nc.sync.dma_start(wt[:], t_w[nm].ap().rearrange("(kt p) o -> p kt o", p=P))
            gate_w[nm] = wt
        wt = pers.tile([P, 2, 256], f16, tag="wgg16")
        nc.sync.dma_start(wt[:], t_w["wgg16"].ap().rearrange("(kt p) o -> p kt o", p=P))
        gate_w["wgg16"] = wt
        wdlx_sb = pers.tile([P, 2], f32r)
        nc.sync.dma_start(wdlx_sb[:], t_w["wdlx"].ap())
        wdlg_sb = pers.tile([P, 2, 2], f32r)
        nc.sync.dma_start(wdlg_sb[:], t_w["wdlg"].ap().rearrange("(kt p) o -> p kt o", p=P))
        rPH_sb = pers.tile([P, 2, B, M], f32)
        nc.sync.dma_start(rPH_sb[:], t_rPH.ap())
        zPH_sb = pers.tile([P, 2, B, M], f32)
        nc.sync.dma_start(zPH_sb[:], t_zPH.ap())
        gPH_sb = pers.tile([P, 2, B, M], f32)
        nc.sync.dma_start(gPH_sb[:], t_gPH.ap())
        dlPH_sb = pers.tile([2, B, M], f32)
        nc.sync.dma_start(dlPH_sb[:], t_dlPH.ap())

        gT16 = pers.tile([P, 2, NPT], f16)
        nc.vector.tensor_copy(gT16[:], gTsb[:])

        def gate_pre(wxname, wgname):
            dest = pers.tile([P, 2, NPT], f16)
            for c in range(2):
                cs = slice(c * P, (c + 1) * P)
                ps = pspool.tile([P, NPT], f32, tag="ps")
                nc.tensor.matmul(ps[:], gate_w[wxname][:, cs], xsd[:],
                                 start=True, stop=False)
                nc.tensor.matmul(ps[:], gate_w[wgname][:, 0, cs], gTsb[:, 0, :],
                                 start=False, stop=False)
                nc.tensor.matmul(ps[:], gate_w[wgname][:, 1, cs], gTsb[:, 1, :],
                                 start=False, stop=True)
                nc.vector.tensor_copy(dest[:, c, :], ps[:])
            return dest

        rN = gate_pre("wrx", "wrg")
        zN = gate_pre("wzx", "wzg")

        run = pers.tile([P, 2, NPT], f16)
        nc.vector.memset(run[:], 0.0)
        for m in range(M):
            r = tpool.tile([P, 2, NPT], f16, tag="r")
            for c in range(2):
                for b in range(B):
                    pts = slice(b * NSH, (b + 1) * NSH)
                    nc.scalar.activation(r[:, c, pts], rN[:, c, pts], AF.Relu,
                                         bias=rPH_sb[:, c, b, m:m + 1])
            rg = tpool.tile([P, 2, NPT], f16, tag="rg")
            for c in range(2):
                nc.vector.tensor_tensor(rg[:, c, :], r[:, c, :], gT16[:, c, :], OP.mult)
            for c in range(2):
                cs = slice(c * P, (c + 1) * P)
                ps = pspool.tile([P, NPT], f32, tag="ps")
                nc.tensor.matmul(ps[:], gate_w["wgx"][:, cs], xsd[:],
                                 start=True, stop=False)
                nc.tensor.matmul(ps[:], gate_w["wgg16"][:, 0, cs], rg[:, 0, :],
                                 start=False, stop=False)
                nc.tensor.matmul(ps[:], gate_w["wgg16"][:, 1, cs], rg[:, 1, :],
                                 start=False, stop=True)
                t = tpool.tile([P, NPT], f16, tag="t")
                for b in range(B):
                    pts = slice(b * NSH, (b + 1) * NSH)
                    nc.scalar.activation(t[:, pts], ps[:, pts], AF.Relu,
                                         bias=gPH_sb[:, c, b, m:m + 1])
                nc.vector.tensor_tensor(run[:, c, :], run[:, c, :], t[:], OP.max)

        diff = pers.tile([P, 2, NPT], f16)
        for c in range(2):
            nc.vector.tensor_tensor(diff[:, c, :], run[:, c, :], gT16[:, c, :],
                                    OP.subtract)

        dxrun = pers.tile([1, NPT], f32)
        nc.vector.memset(dxrun[:], 0.0)
        for m in range(M):
            z = tpool.tile([P, 2, NPT], f16, tag="z")
            for c in range(2):
                for b in range(B):
                    pts = slice(b * NSH, (b + 1) * NSH)
                    nc.vector.tensor_scalar(z[:, c, pts], zN[:, c, pts],
                                            zPH_sb[:, c, b, m:m + 1], 0.0,
                                            OP.add, OP.max)
            gnew = tpool.tile([P, 2, NPT], f32r, tag="gnew")
            t2 = tpool.tile([P, 2, NPT], f16, tag="t2")
            for c in range(2):
                nc.vector.tensor_tensor(t2[:, c, :], z[:, c, :], diff[:, c, :], OP.mult)
                nc.gpsimd.tensor_tensor(gnew[:, c, :], t2[:, c, :], gTsb[:, c, :], OP.add)
            nc.sync.dma_start(o_gnew.ap()[m], gnew[:])
            psd = psdpool.tile([2, NPT], f32, tag="psd")
            nc.tensor.matmul(psd[:], wdlx_sb[:], xsd[:], start=True, stop=False)
            nc.tensor.matmul(psd[:], wdlg_sb[:, 0, :], gnew[:, 0, :],
                             start=False, stop=False)
            nc.tensor.matmul(psd[:], wdlg_sb[:, 1, :], gnew[:, 1, :],
                             start=False, stop=True)
            dl = tpool.tile([2, NPT], f32, tag="dl")
            for b in range(B):
                pts = slice(b * NSH, (b + 1) * NSH)
                nc.scalar.activation(dl[:, pts], psd[:, pts], AF.Relu,
                                     bias=dlPH_sb[:, b, m:m + 1])
            nc.vector.tensor_tensor(dxrun[:], dxrun[:], dl[0:1, :], OP.max)
            nc.sync.dma_start(o_lp.ap()[m:m + 1, :], dl[1:2, :])

        dxred = pers.tile([1, B], f32)
        nc.vector.reduce_max(dxred[:], dxrun[:].rearrange("p (b n) -> p b n", b=B),
                             axis=AX.X)
        nc.sync.dma_start(o_dx.ap(), dxred[:])

    nc.compile()
    return nc


def _get_nc():
    if "nc" not in _CACHE:
        _CACHE["nc"] = _build()
    return _CACHE["nc"]


# --------------------------------------------------------------------------
# gather
# --------------------------------------------------------------------------

def _gather(results):
    f = np.float32
    s = np.empty((B, 1, N, 1), f)
    dsx = np.empty((B, 1, N, 3), f)
    dsh = np.zeros((B, 1, 1, LAT), f)
    Gnew = np.empty((B, M, N, LAT), f)
    lp = np.empty((B, M, N, 1), f)
    dx = np.full((B,), -np.inf, f)
    for c, o in enumerate(results):
        sl = slice(c * NSH, (c + 1) * NSH)
        gnew = o["gnew_out"]            # [M, 128, 2, NPT]
        for b in range(B):
            pts = slice(b * NSH, (b + 1) * NSH)
            s[b, 0, sl, 0] = o["s_out"][0, pts]
            dsx[b, 0, sl, :] = o["dsx_out"][:, pts].T
            dsh[b, 0, 0, :] += o["dsh_out"][:, :, b].T.reshape(LAT)
            lp[b, :, sl, 0] = o["lp_out"][:, pts]
            dx[b] = max(dx[b], o["dx_out"][0, b])
            blk = gnew[:, :, :, pts]    # [M, 128, 2, NSH]
            Gnew[b, :, sl, :] = blk.transpose(0, 3, 2, 1).reshape(M, NSH, LAT)
    return s, dsx, dsh, Gnew, dx.reshape(B, 1, 1, 1), lp


# --------------------------------------------------------------------------
# entry point
# --------------------------------------------------------------------------

def kernel(X, H, P, G, params, _trace=False):
    d, shards = _prep(X, H, P, G, params)
    nc = _get_nc()

    base = {k: np.ascontiguousarray(v, np.float16 if k == "wgg16" else np.float32)
            for k, v in d.items()}
    in_maps = []
    for sh in shards:
        im = dict(base)
        im["xT"] = sh["xT"]
        im["gT"] = sh["gT"]
        in_maps.append(im)

    from concourse.bass_utils import run_bass_kernel_spmd
    res = run_bass_kernel_spmd(nc, in_maps, core_ids=list(range(NCORES)),
                               trace=_trace)
    out = _gather(res.results)
    if _trace:
        return out, res
    return out
